# revision 7
# baseline (speedup 1.0000x reference)
import sys
import numpy as np

sys.path.insert(0, "/opt/trn_rl_repo")

import concourse.bass as bass
import concourse.bacc as bacc
import concourse.mybir as mybir
import concourse.tile as tile
from concourse.bass_utils import run_bass_kernel_spmd

DIM = 128
HID = 256
NB = 2048
NTOT = 32768
NMAX = 32
VAL_MID = 192
SIZE_MID = 128
DEC_MID = 192
NC = 8
BPC = NB // NC          # 256 sets per core
PAD = 4608              # padded element count per core (36 x 128)
NT = PAD // 128         # 36 row tiles
F32 = mybir.dt.float32
I32 = mybir.dt.int32
AF = mybir.ActivationFunctionType
AX = mybir.AxisListType
HALF_PI = float(np.pi / 2)
TWO_PI = float(2 * np.pi)

_CACHE = {}


def _build():
    nc = bacc.Bacc(None, target_bir_lowering=False)

    # per-core inputs
    xsT = nc.dram_tensor("xsT", [DIM, PAD], F32, kind="ExternalInput")
    segT = nc.dram_tensor("segT", [PAD, BPC], F32, kind="ExternalInput")
    nvec = nc.dram_tensor("nvec", [BPC, 1], F32, kind="ExternalInput")
    # replicated params / precomputed broadcast tiles
    w1 = nc.dram_tensor("w1", [DIM, VAL_MID], F32, kind="ExternalInput")
    b1b = nc.dram_tensor("b1b", [128, VAL_MID], F32, kind="ExternalInput")
    g1b = nc.dram_tensor("g1b", [128, VAL_MID], F32, kind="ExternalInput")
    bln1 = nc.dram_tensor("bln1", [128, VAL_MID], F32, kind="ExternalInput")
    w2a = nc.dram_tensor("w2a", [128, HID], F32, kind="ExternalInput")
    w2b = nc.dram_tensor("w2b", [64, HID], F32, kind="ExternalInput")
    b2b = nc.dram_tensor("b2b", [128, HID], F32, kind="ExternalInput")
    cardw = nc.dram_tensor("cardw", [128, HID], F32, kind="ExternalInput")
    cardb = nc.dram_tensor("cardb", [128, HID], F32, kind="ExternalInput")
    ecos = nc.dram_tensor("ecos", [PAD, HID], F32, kind="ExternalInput")
    esin = nc.dram_tensor("esin", [PAD, HID], F32, kind="ExternalInput")
    ident = nc.dram_tensor("ident", [128, 128], F32, kind="ExternalInput")
    negkk = nc.dram_tensor("negkk", [128, NMAX], F32, kind="ExternalInput")
    sw1a = nc.dram_tensor("sw1a", [128, SIZE_MID], F32, kind="ExternalInput")
    sw1b = nc.dram_tensor("sw1b", [128, SIZE_MID], F32, kind="ExternalInput")
    sb1b = nc.dram_tensor("sb1b", [128, SIZE_MID], F32, kind="ExternalInput")
    sgb = nc.dram_tensor("sgb", [128, SIZE_MID], F32, kind="ExternalInput")
    sbb = nc.dram_tensor("sbb", [128, SIZE_MID], F32, kind="ExternalInput")
    sw2 = nc.dram_tensor("sw2", [SIZE_MID, 1], F32, kind="ExternalInput")
    sb2b = nc.dram_tensor("sb2b", [128, 1], F32, kind="ExternalInput")
    dw1a = nc.dram_tensor("dw1a", [128, DEC_MID], F32, kind="ExternalInput")
    dw1b = nc.dram_tensor("dw1b", [128, DEC_MID], F32, kind="ExternalInput")
    db1b = nc.dram_tensor("db1b", [128, DEC_MID], F32, kind="ExternalInput")
    dw2a = nc.dram_tensor("dw2a", [128, DIM], F32, kind="ExternalInput")
    dw2b = nc.dram_tensor("dw2b", [64, DIM], F32, kind="ExternalInput")
    db2b = nc.dram_tensor("db2b", [128, DIM], F32, kind="ExternalInput")
    cosall = nc.dram_tensor("cosall", [NMAX * 128, HID], F32, kind="ExternalInput")
    sinall = nc.dram_tensor("sinall", [NMAX * 128, HID], F32, kind="ExternalInput")
    # outputs
    xr = nc.dram_tensor("xr", [BPC, NMAX, DIM], F32, kind="ExternalOutput")
    npred_o = nc.dram_tensor("npred", [BPC, 1], F32, kind="ExternalOutput")

    with tile.TileContext(nc) as tc:
        def emit_mish(pool, out, x, fdim, tag):
            xc = pool.tile([128, fdim], F32, tag=tag + "xc")
            nc.vector.tensor_scalar_min(xc[:], x[:], 20.0)
            u = pool.tile([128, fdim], F32, tag=tag + "u")
            nc.scalar.activation(u[:], xc[:], AF.Exp)
            nc.vector.tensor_scalar_add(u[:], u[:], 1.0)
            w = pool.tile([128, fdim], F32, tag=tag + "w")
            nc.scalar.activation(w[:], u[:], AF.Square)
            num = pool.tile([128, fdim], F32, tag=tag + "n")
            nc.vector.tensor_scalar_add(num[:], w[:], -1.0)
            nc.vector.tensor_scalar_add(w[:], w[:], 1.0)
            r = pool.tile([128, fdim], F32, tag=tag + "r")
            nc.vector.reciprocal(r[:], w[:])
            nc.vector.tensor_mul(num[:], num[:], r[:])
            nc.vector.tensor_mul(out[:], x[:], num[:])
        with tc.tile_pool(name="const", bufs=1) as cp, \
             tc.tile_pool(name="persist", bufs=1) as pp:
            def cload(dt_, shape, tag):
                t = cp.tile(shape, F32, tag=tag)
                nc.sync.dma_start(t[:], dt_[:])
                return t
            w1_s = cload(w1, [DIM, VAL_MID], "w1")
            b1b_s = cload(b1b, [128, VAL_MID], "b1b")
            g1b_s = cload(g1b, [128, VAL_MID], "g1b")
            bln1_s = cload(bln1, [128, VAL_MID], "bln1")
            w2a_s = cload(w2a, [128, HID], "w2a")
            w2b_s = cload(w2b, [64, HID], "w2b")
            b2b_s = cload(b2b, [128, HID], "b2b")
            cardw_s = cload(cardw, [128, HID], "cardw")
            cardb_s = cload(cardb, [128, HID], "cardb")
            id_s = cload(ident, [128, 128], "ident")
            negkk_s = cload(negkk, [128, NMAX], "negkk")
            sw1a_s = cload(sw1a, [128, SIZE_MID], "sw1a")
            sw1b_s = cload(sw1b, [128, SIZE_MID], "sw1b")
            sb1b_s = cload(sb1b, [128, SIZE_MID], "sb1b")
            sgb_s = cload(sgb, [128, SIZE_MID], "sgb")
            sbb_s = cload(sbb, [128, SIZE_MID], "sbb")
            sw2_s = cload(sw2, [SIZE_MID, 1], "sw2")
            sb2b_s = cload(sb2b, [128, 1], "sb2b")
            dw1a_s = cload(dw1a, [128, DEC_MID], "dw1a")
            dw1b_s = cload(dw1b, [128, DEC_MID], "dw1b")
            db1b_s = cload(db1b, [128, DEC_MID], "db1b")
            dw2a_s = cload(dw2a, [128, DIM], "dw2a")
            dw2b_s = cload(dw2b, [64, DIM], "dw2b")
            db2b_s = cload(db2b, [128, DIM], "db2b")
            eps_t = cp.tile([128, 1], F32, tag="eps")
            nc.vector.memset(eps_t[:], 1e-5)
            neg1_t = cp.tile([128, 1], F32, tag="neg1")
            nc.vector.memset(neg1_t[:], -1.0)
            cos_s = []
            sin_s = []
            for kk in range(NMAX):
                ct = cp.tile([128, HID], F32, tag=f"cos{kk}")
                nc.sync.dma_start(ct[:], cosall[kk * 128:(kk + 1) * 128, :])
                cos_s.append(ct)
                st_ = cp.tile([128, HID], F32, tag=f"sin{kk}")
                nc.sync.dma_start(st_[:], sinall[kk * 128:(kk + 1) * 128, :])
                sin_s.append(st_)

            # persistent z (sbuf) per set-block
            zre_s = [pp.tile([128, HID], F32, tag=f"zre{b}", name=f"zre{b}") for b in range(2)]
            zim_s = [pp.tile([128, HID], F32, tag=f"zim{b}", name=f"zim{b}") for b in range(2)]
            nv_s = [pp.tile([128, 1], F32, tag=f"nv{b}", name=f"nv{b}") for b in range(2)]
            for b in range(2):
                nc.sync.dma_start(nv_s[b][:], nvec[b * 128:(b + 1) * 128, :])

            # ---------------- encoder ----------------
            with tc.tile_pool(name="zp", bufs=1, space="PSUM") as zp, \
                 tc.tile_pool(name="mp", bufs=1, space="PSUM") as mp, \
                 tc.tile_pool(name="wk", bufs=2) as wk:
                zre_p = [zp.tile([128, HID], F32, tag=f"pzre{b}", name=f"pzre{b}") for b in range(2)]
                zim_p = [zp.tile([128, HID], F32, tag=f"pzim{b}", name=f"pzim{b}") for b in range(2)]
                for i in range(NT):
                    xt = wk.tile([DIM, 128], F32, tag="xt")
                    nc.sync.dma_start(xt[:], xsT[:, i * 128:(i + 1) * 128])
                    st = wk.tile([128, BPC], F32, tag="st")
                    nc.sync.dma_start(st[:], segT[i * 128:(i + 1) * 128, :])
                    kcos = wk.tile([128, HID], F32, tag="kcos")
                    nc.sync.dma_start(kcos[:], ecos[i * 128:(i + 1) * 128, :])
                    ksin = wk.tile([128, HID], F32, tag="ksin")
                    nc.sync.dma_start(ksin[:], esin[i * 128:(i + 1) * 128, :])

                    ph1 = mp.tile([128, VAL_MID], F32, tag="ph1")
                    nc.tensor.matmul(ph1[:], xt[:], w1_s[:], start=True, stop=True)
                    a = wk.tile([128, VAL_MID], F32, tag="a")
                    nc.vector.tensor_add(a[:], ph1[:], b1b_s[:])
                    # layernorm over 192 features
                    nm = wk.tile([128, 1], F32, tag="nm")
                    nc.vector.reduce_sum(nm[:], a[:], axis=AX.X)
                    nc.vector.tensor_scalar_mul(nm[:], nm[:], -1.0 / VAL_MID)
                    xm = wk.tile([128, VAL_MID], F32, tag="xm")
                    nc.vector.tensor_scalar_add(xm[:], a[:], nm[:])
                    sq = wk.tile([128, VAL_MID], F32, tag="sq")
                    nc.scalar.activation(sq[:], xm[:], AF.Square)
                    vs = wk.tile([128, 1], F32, tag="vs")
                    nc.vector.reduce_sum(vs[:], sq[:], axis=AX.X)
                    lg = wk.tile([128, 1], F32, tag="lg")
                    nc.scalar.activation(lg[:], vs[:], AF.Ln,
                                         scale=1.0 / VAL_MID, bias=eps_t[:])
                    rs = wk.tile([128, 1], F32, tag="rs")
                    nc.scalar.activation(rs[:], lg[:], AF.Exp, scale=-0.5)
                    xh = wk.tile([128, VAL_MID], F32, tag="xh")
                    nc.vector.tensor_scalar_mul(xh[:], xm[:], rs[:])
                    nc.vector.tensor_mul(xh[:], xh[:], g1b_s[:])
                    nc.vector.tensor_add(xh[:], xh[:], bln1_s[:])
                    hm = wk.tile([128, VAL_MID], F32, tag="hm")
                    emit_mish(wk, hm, xh, VAL_MID, "em")
                    # transpose h -> hT (192 = 128 + 64)
                    pt1 = mp.tile([128, 128], F32, tag="pt1")
                    nc.tensor.transpose(pt1[:], hm[:, 0:128], id_s[:])
                    hT1 = wk.tile([128, 128], F32, tag="hT1")
                    nc.vector.tensor_copy(hT1[:], pt1[:])
                    pt2 = mp.tile([64, 128], F32, tag="pt2")
                    nc.tensor.transpose(pt2[:], hm[:, 128:192], id_s[:])
                    hT2 = wk.tile([64, 128], F32, tag="hT2")
                    nc.vector.tensor_copy(hT2[:], pt2[:])
                    ph2 = mp.tile([128, HID], F32, tag="ph2")
                    nc.tensor.matmul(ph2[:], hT1[:], w2a_s[:], start=True, stop=False)
                    nc.tensor.matmul(ph2[:], hT2[:], w2b_s[:], start=False, stop=True)
                    h2 = wk.tile([128, HID], F32, tag="h2")
                    nc.vector.tensor_add(h2[:], ph2[:], b2b_s[:])
                    yre = wk.tile([128, HID], F32, tag="yre")
                    nc.vector.tensor_mul(yre[:], h2[:], kcos[:])
                    yim = wk.tile([128, HID], F32, tag="yim")
                    nc.vector.tensor_mul(yim[:], h2[:], ksin[:])
                    # segment sum via indicator matmul, accumulated in psum
                    first = (i == 0)
                    last = (i == NT - 1)
                    for b in range(2):
                        nc.tensor.matmul(zre_p[b][:], st[:, b * 128:(b + 1) * 128],
                                         yre[:], start=first, stop=last)
                        nc.tensor.matmul(zim_p[b][:], st[:, b * 128:(b + 1) * 128],
                                         yim[:], start=first, stop=last)
                # z = z_el + n*cardw + cardb (real part only)
                for b in range(2):
                    tmp = wk.tile([128, HID], F32, tag="tmp")
                    nc.vector.tensor_scalar_mul(tmp[:], cardw_s[:], nv_s[b][:])
                    nc.vector.tensor_add(tmp[:], tmp[:], cardb_s[:])
                    nc.vector.tensor_add(zre_s[b][:], zre_p[b][:], tmp[:])
                    nc.vector.tensor_copy(zim_s[b][:], zim_p[b][:])

            # ---------------- decoder ----------------
            with tc.tile_pool(name="dp", bufs=1, space="PSUM") as dp, \
                 tc.tile_pool(name="wk2", bufs=2) as wk2:
                for b in range(2):
                    # size MLP on z.real
                    ptA = dp.tile([128, 128], F32, tag="ptA")
                    zT1 = wk2.tile([128, 128], F32, tag="zT1")
                    nc.tensor.transpose(ptA[:], zre_s[b][:, 0:128], id_s[:])
                    nc.vector.tensor_copy(zT1[:], ptA[:])
                    ptB = dp.tile([128, 128], F32, tag="ptB")
                    zT2 = wk2.tile([128, 128], F32, tag="zT2")
                    nc.tensor.transpose(ptB[:], zre_s[b][:, 128:256], id_s[:])
                    nc.vector.tensor_copy(zT2[:], ptB[:])
                    ps1 = dp.tile([128, SIZE_MID], F32, tag="ps1")
                    nc.tensor.matmul(ps1[:], zT1[:], sw1a_s[:], start=True, stop=False)
                    nc.tensor.matmul(ps1[:], zT2[:], sw1b_s[:], start=False, stop=True)
                    a2 = wk2.tile([128, SIZE_MID], F32, tag="a2")
                    nc.vector.tensor_add(a2[:], ps1[:], sb1b_s[:])
                    nm2 = wk2.tile([128, 1], F32, tag="nm2")
                    nc.vector.reduce_sum(nm2[:], a2[:], axis=AX.X)
                    nc.vector.tensor_scalar_mul(nm2[:], nm2[:], -1.0 / SIZE_MID)
                    xm2 = wk2.tile([128, SIZE_MID], F32, tag="xm2")
                    nc.vector.tensor_scalar_add(xm2[:], a2[:], nm2[:])
                    sq2 = wk2.tile([128, SIZE_MID], F32, tag="sq2")
                    nc.scalar.activation(sq2[:], xm2[:], AF.Square)
                    vs2 = wk2.tile([128, 1], F32, tag="vs2")
                    nc.vector.reduce_sum(vs2[:], sq2[:], axis=AX.X)
                    lg2 = wk2.tile([128, 1], F32, tag="lg2")
                    nc.scalar.activation(lg2[:], vs2[:], AF.Ln,
                                         scale=1.0 / SIZE_MID, bias=eps_t[:])
                    rs2 = wk2.tile([128, 1], F32, tag="rs2")
                    nc.scalar.activation(rs2[:], lg2[:], AF.Exp, scale=-0.5)
                    xh2 = wk2.tile([128, SIZE_MID], F32, tag="xh2")
                    nc.vector.tensor_scalar_mul(xh2[:], xm2[:], rs2[:])
                    nc.vector.tensor_mul(xh2[:], xh2[:], sgb_s[:])
                    nc.vector.tensor_add(xh2[:], xh2[:], sbb_s[:])
                    hm2 = wk2.tile([128, SIZE_MID], F32, tag="hm2")
                    emit_mish(wk2, hm2, xh2, SIZE_MID, "sm")
                    ptC = dp.tile([128, 128], F32, tag="ptA")
                    mT = wk2.tile([128, 128], F32, tag="mT")
                    nc.tensor.transpose(ptC[:], hm2[:], id_s[:])
                    nc.vector.tensor_copy(mT[:], ptC[:])
                    ps2 = dp.tile([128, 1], F32, tag="ps2")
                    nc.tensor.matmul(ps2[:], mT[:], sw2_s[:], start=True, stop=True)
                    s2 = wk2.tile([128, 1], F32, tag="s2")
                    nc.vector.tensor_add(s2[:], ps2[:], sb2b_s[:])
                    # n_pred = max(round(s2), 0) — round via f32->i32->f32 cast
                    ri = wk2.tile([128, 1], I32, tag="ri")
                    nc.vector.tensor_copy(ri[:], s2[:])
                    npf = wk2.tile([128, 1], F32, tag="npf")
                    nc.vector.tensor_copy(npf[:], ri[:])
                    nc.vector.tensor_scalar_max(npf[:], npf[:], 0.0)
                    nc.sync.dma_start(npred_o[b * 128:(b + 1) * 128, :], npf[:])
                    # mask matrix [128 sets, 32]: relu(min(np,32)-kk) - relu(...-1)
                    np32 = wk2.tile([128, 1], F32, tag="np32")
                    nc.vector.tensor_scalar_min(np32[:], npf[:], float(NMAX))
                    dmask = wk2.tile([128, NMAX], F32, tag="dmask")
                    nc.vector.tensor_scalar_add(dmask[:], negkk_s[:], np32[:])
                    mk1 = wk2.tile([128, NMAX], F32, tag="mk1")
                    nc.scalar.activation(mk1[:], dmask[:], AF.Relu)
                    mk2 = wk2.tile([128, NMAX], F32, tag="mk2")
                    nc.scalar.activation(mk2[:], dmask[:], AF.Relu, bias=neg1_t[:])
                    mkm = wk2.tile([128, NMAX], F32, tag="mkm")
                    nc.vector.tensor_sub(mkm[:], mk1[:], mk2[:])
                    # zc = z - (n_pred*cardw + cardb) (real only)
                    tmp2 = wk2.tile([128, HID], F32, tag="tmp2")
                    nc.vector.tensor_scalar_mul(tmp2[:], cardw_s[:], npf[:])
                    nc.vector.tensor_add(tmp2[:], tmp2[:], cardb_s[:])
                    zcre = wk2.tile([128, HID], F32, tag="zcre")
                    nc.vector.tensor_sub(zcre[:], zre_s[b][:], tmp2[:])
                    # decoder broadcast over NMAX positions
                    for kk in range(NMAX):
                        zpre = wk2.tile([128, HID], F32, tag="zpre")
                        nc.vector.tensor_mul(zpre[:], zcre[:], cos_s[kk][:])
                        zpt = wk2.tile([128, HID], F32, tag="zpt")
                        nc.vector.tensor_mul(zpt[:], zim_s[b][:], sin_s[kk][:])
                        nc.vector.tensor_sub(zpre[:], zpre[:], zpt[:])
                        ptD = dp.tile([128, 128], F32, tag="ptA")
                        zpT1 = wk2.tile([128, 128], F32, tag="zpT1")
                        nc.tensor.transpose(ptD[:], zpre[:, 0:128], id_s[:])
                        nc.vector.tensor_copy(zpT1[:], ptD[:])
                        ptE = dp.tile([128, 128], F32, tag="ptB")
                        zpT2 = wk2.tile([128, 128], F32, tag="zpT2")
                        nc.tensor.transpose(ptE[:], zpre[:, 128:256], id_s[:])
                        nc.vector.tensor_copy(zpT2[:], ptE[:])
                        pdh = dp.tile([128, DEC_MID], F32, tag="pdh")
                        nc.tensor.matmul(pdh[:], zpT1[:], dw1a_s[:], start=True, stop=False)
                        nc.tensor.matmul(pdh[:], zpT2[:], dw1b_s[:], start=False, stop=True)
                        hd = wk2.tile([128, DEC_MID], F32, tag="hd")
                        nc.vector.tensor_add(hd[:], pdh[:], db1b_s[:])
                        hdm = wk2.tile([128, DEC_MID], F32, tag="hdm")
                        emit_mish(wk2, hdm, hd, DEC_MID, "dm")
                        ptF = dp.tile([128, 128], F32, tag="ptA")
                        hdT1 = wk2.tile([128, 128], F32, tag="hdT1")
                        nc.tensor.transpose(ptF[:], hdm[:, 0:128], id_s[:])
                        nc.vector.tensor_copy(hdT1[:], ptF[:])
                        ptG = dp.tile([64, 128], F32, tag="ptB")
                        hdT2 = wk2.tile([64, 128], F32, tag="hdT2")
                        nc.tensor.transpose(ptG[:], hdm[:, 128:192], id_s[:])
                        nc.vector.tensor_copy(hdT2[:], ptG[:])
                        pdo = dp.tile([128, DIM], F32, tag="pdo")
                        nc.tensor.matmul(pdo[:], hdT1[:], dw2a_s[:], start=True, stop=False)
                        nc.tensor.matmul(pdo[:], hdT2[:], dw2b_s[:], start=False, stop=True)
                        xo = wk2.tile([128, DIM], F32, tag="xo")
                        nc.vector.tensor_add(xo[:], pdo[:], db2b_s[:])
                        xom = wk2.tile([128, DIM], F32, tag="xom")
                        nc.vector.tensor_scalar_mul(xom[:], xo[:], mkm[:, kk:kk + 1])
                        nc.sync.dma_start(xr[b * 128:(b + 1) * 128, kk, :], xom[:])
    nc.compile()
    return nc


def kernel(x, batch, rank_w, rank_b, val_w1, val_b1, val_ln_g, val_ln_b,
           val_w2, val_b2, card_w, card_b, size_w1, size_b1, size_ln_g,
           size_ln_b, size_w2, size_b2, dec_w1, dec_b1, dec_w2, dec_b2):
    x = np.asarray(x, np.float32)
    batch = np.asarray(batch).astype(np.int64)
    bi = batch.astype(np.int32)

    # host-side index prep: rank scores + stable per-set sort
    mag = x @ np.asarray(rank_w, np.float32) + np.float32(rank_b)
    order = np.lexsort((mag, bi))
    xs = x[order]
    n = np.bincount(bi, minlength=NB).astype(np.int64)
    seg_start = np.cumsum(n) - n
    k_all = (np.arange(NTOT, dtype=np.int64) - seg_start[bi]).astype(np.float32)

    rep = lambda v, p=128: np.broadcast_to(np.asarray(v, np.float32)[None, :],
                                           (p, len(v))).copy()
    t = np.linspace(0.0, 1.0, HID).astype(np.float32)
    kk = np.arange(NMAX, dtype=np.float32)
    arg2 = (t[None, :] * kk[:, None] * 8.0).astype(np.float32)
    cosall = np.repeat(np.cos(arg2), 128, axis=0).astype(np.float32)
    sinall = np.repeat(np.sin(arg2), 128, axis=0).astype(np.float32)

    common = {
        "w1": np.asarray(val_w1, np.float32),
        "b1b": rep(val_b1), "g1b": rep(val_ln_g), "bln1": rep(val_ln_b),
        "w2a": np.asarray(val_w2[0:128], np.float32),
        "w2b": np.asarray(val_w2[128:192], np.float32),
        "b2b": rep(val_b2),
        "cardw": rep(np.asarray(card_w, np.float32).reshape(-1)),
        "cardb": rep(card_b),
        "ident": np.eye(128, dtype=np.float32),
        "negkk": rep(-kk),
        "sw1a": np.asarray(size_w1[0:128], np.float32),
        "sw1b": np.asarray(size_w1[128:256], np.float32),
        "sb1b": rep(size_b1), "sgb": rep(size_ln_g), "sbb": rep(size_ln_b),
        "sw2": np.asarray(size_w2, np.float32).reshape(SIZE_MID, 1),
        "sb2b": np.full((128, 1), np.float32(np.asarray(size_b2).reshape(-1)[0])),
        "dw1a": np.asarray(dec_w1[0:128], np.float32),
        "dw1b": np.asarray(dec_w1[128:256], np.float32),
        "db1b": rep(dec_b1),
        "dw2a": np.asarray(dec_w2[0:128], np.float32),
        "dw2b": np.asarray(dec_w2[128:192], np.float32),
        "db2b": rep(dec_b2),
        "cosall": cosall, "sinall": sinall,
    }

    in_maps = []
    for c in range(NC):
        b0 = c * BPC
        s = int(seg_start[b0])
        e = int(seg_start[b0 + BPC - 1] + n[b0 + BPC - 1]) if c == NC - 1 or True else 0
        cnt = e - s
        assert cnt <= PAD, f"shard {c} has {cnt} elements > PAD {PAD}"
        xsT_p = np.zeros((DIM, PAD), np.float32)
        xsT_p[:, :cnt] = xs[s:e].T
        argk = 8.0 * t[None, :].astype(np.float64) * k_all[s:e, None].astype(np.float64)
        ecos_p = np.zeros((PAD, HID), np.float32)
        ecos_p[:cnt] = np.cos(argk)
        esin_p = np.zeros((PAD, HID), np.float32)
        esin_p[:cnt] = np.sin(argk)
        segT_p = np.zeros((PAD, BPC), np.float32)
        segT_p[np.arange(cnt), bi[order][s:e] - b0] = 1.0
        nvec_p = n[b0:b0 + BPC].astype(np.float32).reshape(BPC, 1)
        m = {"xsT": xsT_p, "segT": segT_p, "nvec": nvec_p,
             "ecos": ecos_p, "esin": esin_p}
        m.update(common)
        in_maps.append(m)

    if "nc" not in _CACHE:
        _CACHE["nc"] = _build()
    res = run_bass_kernel_spmd(_CACHE["nc"], in_maps, core_ids=list(range(NC)))
    kernel.last_results = res

    xr = np.concatenate([np.asarray(r["xr"]) for r in res.results], axis=0)
    npred = np.concatenate([np.asarray(r["npred"]) for r in res.results],
                           axis=0).reshape(NB)
    n_pred_i = np.minimum(npred.astype(np.int32), NMAX)
    mask = np.arange(NMAX, dtype=np.int32)[None, :] < n_pred_i[:, None]
    return xr, mask, n_pred_i
